# revision 25
# baseline (speedup 1.0000x reference)
"""Deformable conv 3x3 (B=4, C=256, H=W=64, Cout=256) on 8 trn2 NeuronCores.

Sharding: data-parallel — core i handles batch i//2, output-row half i%2
(32 rows = 2048 output positions per core); weight replicated.

Per-core device pipeline (v2, all shapes hardcoded for this problem):
  1. Host precomputes a zero-padded "bilinear basis" image per batch:
     for each padded pixel p=(y,x): [A, C, B, E] x 256ch fp16 where
     A=x[y,x], C=x[y+1,x]-A, B=x[y,x+1]-A, E=x[y+1,x+1]-x[y+1,x]-x[y,x+1]+A.
     Bilinear sample == A + dx*B + dy*C + dx*dy*E == (A+dx*B) + dy*(C+dx*E),
     with zero padding reproducing the reference's out-of-image masking.
  2. dma_gather (SWDGE): per (tap, jblock of 512 positions) gather the 2KB
     basis row of each sample point -> R [128 j, 4 jc, 1024] fp16.
  3. Combine on DVE: 2 fused scalar_tensor_tensor ops per (tap, jc):
     t = (BE * dx) + AC   (contiguous [128,512])
     G = (t_hi * dy) + t_lo  ([128,256])    -> G [128 j, 4 jc, 9*256]
  4. PE transpose (identity matmul) G -> rhs [c_kk, j] fp16; PSUM->SBUF
     copies on the Scalar engine.
  5. GEMM: out[o, j] = sum_{c,kk} W[(kk,c), o] * rhs[(kk,c), j], fp32 PSUM,
     K = 2304 (18 chunks), M = 256 (2 chunks), N = 512 per jblock.

kernel(x, offset, weight) takes full fp32 inputs, returns [4,256,64,64] fp32.
"""
import numpy as np
from contextlib import ExitStack

import concourse.bass as bass
import concourse.bacc as bacc
import concourse.tile as tile
from concourse import mybir
from concourse.bass_utils import run_bass_kernel_spmd

# ---------------------------------------------------------------- constants
B, C, H, W = 4, 256, 64, 64
COUT = 256
K = 3
KK = 9
NCORES = 8
ROWS = 32              # output rows per core
J = ROWS * W           # 2048 output positions per core
JBLK = 4               # jblocks
JB = J // JBLK         # 512
JC = JB // 128         # 4
KCH = (C * KK) // 128  # 18 contraction chunks
MCH = COUT // 128      # 2
NIDX = JB              # indices per gather
NCOL = NIDX // 16      # idx columns per gather block
PADM = 8               # padding margin (covers |offset| < 7)
HP = H + 2 * PADM      # 80
WP = W + 2 * PADM      # 80

DT = mybir.dt.float16
NPDT = np.float16
F32 = mybir.dt.float32


# ---------------------------------------------------------------- host prep
def _make_basis_layout(xb):
    """xb [C,H,W] fp32 -> L [HP*WP, 4*C] fp16 basis rows [A, C, B, E]."""
    xp = np.zeros((HP, WP, C), np.float32)
    xp[PADM:PADM + H, PADM:PADM + W] = xb.transpose(1, 2, 0)
    out = np.zeros((HP, WP, 4, C), np.float32)
    a = xp[:-1, :-1]
    out[:-1, :-1, 0] = a                                # A
    out[:-1, :-1, 1] = xp[1:, :-1] - a                  # C (dy term)
    out[:-1, :-1, 2] = xp[:-1, 1:] - a                  # B (dx term)
    out[:-1, :-1, 3] = xp[1:, 1:] - xp[1:, :-1] - xp[:-1, 1:] + a  # E
    return out.reshape(HP * WP, 4 * C).astype(NPDT)


def _make_idx_w(offset_b, h0):
    """-> idx [KK, J] int16 (padded-grid row), w [KK, J, 2] fp32 (dx, dy)."""
    off = offset_b.reshape(KK, 2, H, W)
    ho = np.arange(h0, h0 + ROWS, dtype=np.float32)
    wo = np.arange(W, dtype=np.float32)
    ky = np.repeat(np.arange(K, dtype=np.float32), K)
    kx = np.tile(np.arange(K, dtype=np.float32), K)
    py = ho[None, :, None] + ky[:, None, None] - 1.0 + off[:, 0, h0:h0 + ROWS, :]
    px = wo[None, None, :] + kx[:, None, None] - 1.0 + off[:, 1, h0:h0 + ROWS, :]
    y0f = np.floor(py)
    x0f = np.floor(px)
    dy = (py - y0f).astype(np.float32)
    dx = (px - x0f).astype(np.float32)
    yi = np.clip(y0f.astype(np.int64) + PADM, 0, HP - 2)
    xi = np.clip(x0f.astype(np.int64) + PADM, 0, WP - 2)
    idx = (yi * WP + xi).astype(np.int16)
    w = np.stack([dx, dy], axis=-1)
    return idx.reshape(KK, J), w.reshape(KK, J, 2)


def _pack_idx(idx):
    """[KK, J] -> [128, KK*JBLK*NCOL] int16; gather g=(t,jb) slice
    [:, g*NCOL:(g+1)*NCOL], idx i at [i%16, i//16], replicated to 8 groups
    of 16 partitions (each GPSIMD Q7 core reads its own group)."""
    out = np.zeros((16, KK * JBLK * NCOL), np.int16)
    for t in range(KK):
        for jb in range(JBLK):
            g = t * JBLK + jb
            v = idx[t, jb * JB:(jb + 1) * JB]
            out[:, g * NCOL:(g + 1) * NCOL] = v.reshape(NCOL, 16).T
    return np.tile(out, (8, 1))


def _pack_w(w):
    """[KK, J, 2] -> [128, KK*JBLK*JC*2] fp32; col ((t*JBLK+jb)*JC+jc)*2+s."""
    a = w.reshape(KK, JBLK, JC, 128, 2)
    return np.ascontiguousarray(
        a.transpose(3, 0, 1, 2, 4).reshape(128, KK * JBLK * JC * 2))


def _pack_weight(weight):
    """[COUT, C, 3, 3] fp32 -> [128, KCH*COUT] fp16; K-order kk*C+c,
    lhsT tile (kc, m) at cols [kc*COUT + m*128, +128)."""
    wm = weight.reshape(COUT, C, KK).transpose(2, 1, 0).reshape(KK * C, COUT)
    wm = wm.reshape(KCH, 128, COUT).transpose(1, 0, 2).reshape(128, KCH * COUT)
    return np.ascontiguousarray(wm).astype(NPDT)


# ---------------------------------------------------------------- program
_PROG = None


def _build_program(dbg=False):
    nc = bacc.Bacc(
        "TRN2",
        target_bir_lowering=False,
        debug=False,
        enable_asserts=False,
        num_devices=NCORES,
    )
    L_t = nc.dram_tensor("xbasis", [HP * WP, 4 * C], DT, kind="ExternalInput")
    WL_t = nc.dram_tensor("wmat", [128, KCH * COUT], DT, kind="ExternalInput")
    IDX_t = nc.dram_tensor("idx", [128, KK * JBLK * NCOL], mybir.dt.int16,
                           kind="ExternalInput")
    WSL_t = nc.dram_tensor("wslot", [128, KK * JBLK * JC * 2], F32,
                           kind="ExternalInput")
    ID_t = nc.dram_tensor("ident", [128, 128], DT, kind="ExternalInput")
    OUT_t = nc.dram_tensor("out", [COUT, J], F32, kind="ExternalOutput")
    out_ap = OUT_t.ap()
    if dbg:
        DR_t = nc.dram_tensor("dbg_r", [128, 4 * 4 * C], DT,
                              kind="ExternalOutput")
        DG_t = nc.dram_tensor("dbg_g", [128, JC * KK * C], DT,
                              kind="ExternalOutput")
        DRHS_t = nc.dram_tensor("dbg_rhs", [128, KCH * JB], DT,
                                kind="ExternalOutput")

    src_ap = bass.AP(L_t, 0, [[4 * C, HP * WP], [1, 4 * C]])

    mult = mybir.AluOpType.mult
    add = mybir.AluOpType.add

    with tile.TileContext(nc) as tc, ExitStack() as ctx:
        const = ctx.enter_context(tc.tile_pool(name="const", bufs=1))
        rpool = ctx.enter_context(tc.tile_pool(name="r", bufs=4))
        gpool = ctx.enter_context(tc.tile_pool(name="g", bufs=2))
        rhspool = ctx.enter_context(tc.tile_pool(name="rhs", bufs=2))
        accpool = ctx.enter_context(tc.tile_pool(name="acc", bufs=12))
        outpool = ctx.enter_context(tc.tile_pool(name="osb", bufs=2))
        pst = ctx.enter_context(tc.tile_pool(name="pst", bufs=5, space="PSUM"))
        psm = ctx.enter_context(tc.tile_pool(name="psm", bufs=3, space="PSUM"))

        w_sb = const.tile([128, KCH * COUT], DT)
        nc.sync.dma_start(w_sb[:], WL_t.ap())
        idx_sb = const.tile([128, KK * JBLK * NCOL], mybir.dt.int16)
        nc.sync.dma_start(idx_sb[:], IDX_t.ap())
        wsl_sb = const.tile([128, KK * JBLK * JC * 2], F32)
        nc.sync.dma_start(wsl_sb[:], WSL_t.ap())
        id_sb = const.tile([128, 128], DT)
        nc.sync.dma_start(id_sb[:], ID_t.ap())

        for jb in range(JBLK):
            gt = gpool.tile([128, JC, KK * C], DT)
            for t in range(KK):
                g = t * JBLK + jb
                r = rpool.tile([128, JC, 4 * C], DT, tag="r")
                nc.gpsimd.dma_gather(
                    r[:],
                    src_ap,
                    idx_sb[:, g * NCOL:(g + 1) * NCOL],
                    NIDX,
                    NIDX,
                    4 * C,
                )
                if dbg and jb == 0 and t == 0:
                    nc.sync.dma_start(DR_t.ap()[:, :], r[:])
                for jc in range(JC):
                    cw = (g * JC + jc) * 2
                    dxs = wsl_sb[:, cw:cw + 1]
                    dys = wsl_sb[:, cw + 1:cw + 2]
                    t12 = accpool.tile([128, 2, C], DT, tag="acc")
                    # t12 = [A,C] + dx*[B,E]   (elem order A,C,B,E)
                    nc.vector.scalar_tensor_tensor(
                        t12[:], r[:, jc, 2 * C:4 * C], dxs,
                        r[:, jc, 0:2 * C], mult, add)
                    # G = t12[A'] + dy*t12[C']
                    nc.vector.scalar_tensor_tensor(
                        gt[:, jc, t * C:(t + 1) * C], t12[:, 1, :], dys,
                        t12[:, 0, :], mult, add)

            if dbg and jb == 0:
                nc.sync.dma_start(DG_t.ap()[:, :], gt[:])
            rhs = rhspool.tile([128, KCH, JB], DT)
            for kc in range(KCH):
                ps = pst.tile([128, JB], DT)
                for jc in range(JC):
                    nc.tensor.transpose(ps[:, jc * 128:(jc + 1) * 128],
                                        gt[:, jc, kc * 128:(kc + 1) * 128],
                                        id_sb[:])
                nc.scalar.copy(rhs[:, kc, :], ps[:])

            if dbg and jb == 0:
                nc.sync.dma_start(DRHS_t.ap()[:, :], rhs[:])
            for m in range(MCH):
                pso = psm.tile([128, JB], F32)
                for kc in range(KCH):
                    nc.tensor.matmul(
                        pso[:],
                        w_sb[:, kc * COUT + m * 128:kc * COUT + (m + 1) * 128],
                        rhs[:, kc, :],
                        start=(kc == 0),
                        stop=(kc == KCH - 1),
                    )
                osb = outpool.tile([128, JB], F32)
                nc.scalar.copy(osb[:], pso[:])
                nc.sync.dma_start(
                    out_ap[m * 128:(m + 1) * 128, jb * JB:(jb + 1) * JB],
                    osb[:])

    nc.compile()
    return nc


def _get_program():
    global _PROG
    if _PROG is None:
        _PROG = _build_program()
    return _PROG


# ---------------------------------------------------------------- entry
def make_in_maps(x, offset, weight):
    x = np.asarray(x, np.float32)
    offset = np.asarray(offset, np.float32)
    weight = np.asarray(weight, np.float32)
    WL = _pack_weight(weight)
    ident = np.eye(128, dtype=NPDT)
    basis = [_make_basis_layout(x[b]) for b in range(B)]
    maps = []
    for core in range(NCORES):
        b, half = core // 2, core % 2
        idx, w = _make_idx_w(offset[b], half * ROWS)
        maps.append({
            "xbasis": basis[b],
            "idx": _pack_idx(idx),
            "wslot": _pack_w(w),
            "wmat": WL,
            "ident": ident,
        })
    return maps


def assemble(results):
    full = np.zeros((B, COUT, H, W), np.float32)
    for core in range(NCORES):
        b, half = core // 2, core % 2
        h0 = half * ROWS
        full[b, :, h0:h0 + ROWS, :] = \
            np.asarray(results[core]["out"]).reshape(COUT, ROWS, W)
    return full


def kernel(x, offset, weight):
    nc = _get_program()
    in_maps = make_in_maps(x, offset, weight)
    res = run_bass_kernel_spmd(nc, in_maps, list(range(NCORES)))
    return assemble(res.results)


# revision 26
# speedup vs baseline: 1.0948x; 1.0948x over previous
"""Deformable conv 3x3 (B=4, C=256, H=W=64, Cout=256) on 8 trn2 NeuronCores.

Sharding: data-parallel — core i handles batch i//2, output-row half i%2
(32 rows = 2048 output positions per core); weight replicated.

Per-core device pipeline (v2, all shapes hardcoded for this problem):
  1. Host precomputes a zero-padded "bilinear basis" image per batch:
     for each padded pixel p=(y,x): [A, C, B, E] x 256ch fp16 where
     A=x[y,x], C=x[y+1,x]-A, B=x[y,x+1]-A, E=x[y+1,x+1]-x[y+1,x]-x[y,x+1]+A.
     Bilinear sample == A + dx*B + dy*C + dx*dy*E == (A+dx*B) + dy*(C+dx*E),
     with zero padding reproducing the reference's out-of-image masking.
  2. dma_gather (SWDGE): per (tap, jblock of 512 positions) gather the 2KB
     basis row of each sample point -> R [128 j, 4 jc, 1024] fp16.
  3. Combine on DVE: 2 fused scalar_tensor_tensor ops per (tap, jc):
     t = (BE * dx) + AC   (contiguous [128,512])
     G = (t_hi * dy) + t_lo  ([128,256])    -> G [128 j, 4 jc, 9*256]
  4. PE transpose (identity matmul) G -> rhs [c_kk, j] fp16; PSUM->SBUF
     copies on the Scalar engine.
  5. GEMM: out[o, j] = sum_{c,kk} W[(kk,c), o] * rhs[(kk,c), j], fp32 PSUM,
     K = 2304 (18 chunks), M = 256 (2 chunks), N = 512 per jblock.

kernel(x, offset, weight) takes full fp32 inputs, returns [4,256,64,64] fp32.
"""
import numpy as np
from contextlib import ExitStack

import concourse.bass as bass
import concourse.bacc as bacc
import concourse.tile as tile
from concourse import mybir
from concourse.bass_utils import run_bass_kernel_spmd

# ---------------------------------------------------------------- constants
B, C, H, W = 4, 256, 64, 64
COUT = 256
K = 3
KK = 9
NCORES = 8
ROWS = 32              # output rows per core
J = ROWS * W           # 2048 output positions per core
JBLK = 4               # jblocks
JB = J // JBLK         # 512
JC = JB // 128         # 4
KCH = (C * KK) // 128  # 18 contraction chunks
MCH = COUT // 128      # 2
NIDX = JB              # indices per gather
NCOL = NIDX // 16      # idx columns per gather block
PADM = 8               # padding margin (covers |offset| < 7)
HP = H + 2 * PADM      # 80
WP = W + 2 * PADM      # 80

DT = mybir.dt.float16
NPDT = np.float16
F32 = mybir.dt.float32


# ---------------------------------------------------------------- host prep
def _make_basis_layout(xb):
    """xb [C,H,W] fp32 -> L [HP*WP, 4*C] fp16 basis rows [A, C, B, E]."""
    xp = np.zeros((HP, WP, C), np.float32)
    xp[PADM:PADM + H, PADM:PADM + W] = xb.transpose(1, 2, 0)
    out = np.zeros((HP, WP, 4, C), np.float32)
    a = xp[:-1, :-1]
    out[:-1, :-1, 0] = a                                # A
    out[:-1, :-1, 1] = xp[1:, :-1] - a                  # C (dy term)
    out[:-1, :-1, 2] = xp[:-1, 1:] - a                  # B (dx term)
    out[:-1, :-1, 3] = xp[1:, 1:] - xp[1:, :-1] - xp[:-1, 1:] + a  # E
    return out.reshape(HP * WP, 4 * C).astype(NPDT)


def _make_idx_w(offset_b, h0):
    """-> idx [KK, J] int16 (padded-grid row), w [KK, J, 2] fp32 (dx, dy)."""
    off = offset_b.reshape(KK, 2, H, W)
    ho = np.arange(h0, h0 + ROWS, dtype=np.float32)
    wo = np.arange(W, dtype=np.float32)
    ky = np.repeat(np.arange(K, dtype=np.float32), K)
    kx = np.tile(np.arange(K, dtype=np.float32), K)
    py = ho[None, :, None] + ky[:, None, None] - 1.0 + off[:, 0, h0:h0 + ROWS, :]
    px = wo[None, None, :] + kx[:, None, None] - 1.0 + off[:, 1, h0:h0 + ROWS, :]
    y0f = np.floor(py)
    x0f = np.floor(px)
    dy = (py - y0f).astype(np.float32)
    dx = (px - x0f).astype(np.float32)
    yi = np.clip(y0f.astype(np.int64) + PADM, 0, HP - 2)
    xi = np.clip(x0f.astype(np.int64) + PADM, 0, WP - 2)
    idx = (yi * WP + xi).astype(np.int16)
    w = np.stack([dx, dy], axis=-1)
    return idx.reshape(KK, J), w.reshape(KK, J, 2)


def _pack_idx(idx):
    """[KK, J] -> [128, KK*JBLK*NCOL] int16; gather g=(t,jb) slice
    [:, g*NCOL:(g+1)*NCOL], idx i at [i%16, i//16], replicated to 8 groups
    of 16 partitions (each GPSIMD Q7 core reads its own group)."""
    out = np.zeros((16, KK * JBLK * NCOL), np.int16)
    for t in range(KK):
        for jb in range(JBLK):
            g = t * JBLK + jb
            v = idx[t, jb * JB:(jb + 1) * JB]
            out[:, g * NCOL:(g + 1) * NCOL] = v.reshape(NCOL, 16).T
    return np.tile(out, (8, 1))


def _pack_w(w):
    """[KK, J, 2] -> [128, KK*JBLK*JC*2] fp32; col ((t*JBLK+jb)*JC+jc)*2+s."""
    a = w.reshape(KK, JBLK, JC, 128, 2)
    return np.ascontiguousarray(
        a.transpose(3, 0, 1, 2, 4).reshape(128, KK * JBLK * JC * 2))


def _pack_weight(weight):
    """[COUT, C, 3, 3] fp32 -> [128, KCH*COUT] fp16; K-order kk*C+c,
    lhsT tile (kc, m) at cols [kc*COUT + m*128, +128)."""
    wm = weight.reshape(COUT, C, KK).transpose(2, 1, 0).reshape(KK * C, COUT)
    wm = wm.reshape(KCH, 128, COUT).transpose(1, 0, 2).reshape(128, KCH * COUT)
    return np.ascontiguousarray(wm).astype(NPDT)


# ---------------------------------------------------------------- program
_PROG = None


def _build_program(dbg=False):
    nc = bacc.Bacc(
        "TRN2",
        target_bir_lowering=False,
        debug=False,
        enable_asserts=False,
        num_devices=NCORES,
    )
    L_t = nc.dram_tensor("xbasis", [HP * WP, 4 * C], DT, kind="ExternalInput")
    WL_t = nc.dram_tensor("wmat", [128, KCH * COUT], DT, kind="ExternalInput")
    IDX_t = nc.dram_tensor("idx", [128, KK * JBLK * NCOL], mybir.dt.int16,
                           kind="ExternalInput")
    WSL_t = nc.dram_tensor("wslot", [128, KK * JBLK * JC * 2], F32,
                           kind="ExternalInput")
    ID_t = nc.dram_tensor("ident", [128, 128], DT, kind="ExternalInput")
    OUT_t = nc.dram_tensor("out", [COUT, J], F32, kind="ExternalOutput")
    out_ap = OUT_t.ap()
    if dbg:
        DR_t = nc.dram_tensor("dbg_r", [128, 4 * 4 * C], DT,
                              kind="ExternalOutput")
        DG_t = nc.dram_tensor("dbg_g", [128, JC * KK * C], DT,
                              kind="ExternalOutput")
        DRHS_t = nc.dram_tensor("dbg_rhs", [128, KCH * JB], DT,
                                kind="ExternalOutput")

    src_ap = bass.AP(L_t, 0, [[4 * C, HP * WP], [1, 4 * C]])

    mult = mybir.AluOpType.mult
    add = mybir.AluOpType.add

    with tile.TileContext(nc) as tc, ExitStack() as ctx:
        const = ctx.enter_context(tc.tile_pool(name="const", bufs=1))
        rpool = ctx.enter_context(tc.tile_pool(name="r", bufs=3))
        gpool = ctx.enter_context(tc.tile_pool(name="g", bufs=2))
        rhspool = ctx.enter_context(tc.tile_pool(name="rhs", bufs=2))
        accpool = ctx.enter_context(tc.tile_pool(name="acc", bufs=12))
        outpool = ctx.enter_context(tc.tile_pool(name="osb", bufs=2))
        pst = ctx.enter_context(tc.tile_pool(name="pst", bufs=4, space="PSUM"))
        psm = ctx.enter_context(tc.tile_pool(name="psm", bufs=2, space="PSUM"))

        w_sb = const.tile([128, KCH * COUT], DT)
        nc.sync.dma_start(w_sb[:], WL_t.ap())
        idx_sb = const.tile([128, KK * JBLK * NCOL], mybir.dt.int16)
        nc.sync.dma_start(idx_sb[:], IDX_t.ap())
        wsl_sb = const.tile([128, KK * JBLK * JC * 2], F32)
        nc.sync.dma_start(wsl_sb[:], WSL_t.ap())
        id_sb = const.tile([128, 128], DT)
        nc.sync.dma_start(id_sb[:], ID_t.ap())

        for jb in range(JBLK):
            gt = gpool.tile([128, JC, KK * C], DT)
            for t in range(KK):
                g = t * JBLK + jb
                r = rpool.tile([128, JC, 4 * C], DT, tag="r")
                nc.gpsimd.dma_gather(
                    r[:],
                    src_ap,
                    idx_sb[:, g * NCOL:(g + 1) * NCOL],
                    NIDX,
                    NIDX,
                    4 * C,
                )
                if dbg and jb == 0 and t == 0:
                    nc.sync.dma_start(DR_t.ap()[:, :], r[:])
                for jc in range(JC):
                    cw = (g * JC + jc) * 2
                    dxs = wsl_sb[:, cw:cw + 1]
                    dys = wsl_sb[:, cw + 1:cw + 2]
                    t12 = accpool.tile([128, 2, C], DT, tag="acc")
                    # t12 = [A,C] + dx*[B,E]   (elem order A,C,B,E)
                    nc.vector.scalar_tensor_tensor(
                        t12[:], r[:, jc, 2 * C:4 * C], dxs,
                        r[:, jc, 0:2 * C], mult, add)
                    # G = t12[A'] + dy*t12[C']
                    nc.vector.scalar_tensor_tensor(
                        gt[:, jc, t * C:(t + 1) * C], t12[:, 1, :], dys,
                        t12[:, 0, :], mult, add)

            if dbg and jb == 0:
                nc.sync.dma_start(DG_t.ap()[:, :], gt[:])
            rhs = rhspool.tile([128, KCH, JB], DT)
            for kc in range(KCH):
                ps = pst.tile([128, JB], DT)
                for jc in range(JC):
                    nc.tensor.transpose(ps[:, jc * 128:(jc + 1) * 128],
                                        gt[:, jc, kc * 128:(kc + 1) * 128],
                                        id_sb[:])
                nc.scalar.copy(rhs[:, kc, :], ps[:])

            if dbg and jb == 0:
                nc.sync.dma_start(DRHS_t.ap()[:, :], rhs[:])
            for m in range(MCH):
                pso = psm.tile([128, JB], F32)
                for kc in range(KCH):
                    nc.tensor.matmul(
                        pso[:],
                        w_sb[:, kc * COUT + m * 128:kc * COUT + (m + 1) * 128],
                        rhs[:, kc, :],
                        start=(kc == 0),
                        stop=(kc == KCH - 1),
                    )
                osb = outpool.tile([128, JB], F32)
                nc.scalar.copy(osb[:], pso[:])
                nc.sync.dma_start(
                    out_ap[m * 128:(m + 1) * 128, jb * JB:(jb + 1) * JB],
                    osb[:])

    nc.compile()
    return nc


def _get_program():
    global _PROG
    if _PROG is None:
        _PROG = _build_program()
    return _PROG


# ---------------------------------------------------------------- entry
def make_in_maps(x, offset, weight):
    x = np.asarray(x, np.float32)
    offset = np.asarray(offset, np.float32)
    weight = np.asarray(weight, np.float32)
    WL = _pack_weight(weight)
    ident = np.eye(128, dtype=NPDT)
    basis = [_make_basis_layout(x[b]) for b in range(B)]
    maps = []
    for core in range(NCORES):
        b, half = core // 2, core % 2
        idx, w = _make_idx_w(offset[b], half * ROWS)
        maps.append({
            "xbasis": basis[b],
            "idx": _pack_idx(idx),
            "wslot": _pack_w(w),
            "wmat": WL,
            "ident": ident,
        })
    return maps


def assemble(results):
    full = np.zeros((B, COUT, H, W), np.float32)
    for core in range(NCORES):
        b, half = core // 2, core % 2
        h0 = half * ROWS
        full[b, :, h0:h0 + ROWS, :] = \
            np.asarray(results[core]["out"]).reshape(COUT, ROWS, W)
    return full


def kernel(x, offset, weight):
    nc = _get_program()
    in_maps = make_in_maps(x, offset, weight)
    res = run_bass_kernel_spmd(nc, in_maps, list(range(NCORES)))
    return assemble(res.results)
